# revision 1
# baseline (speedup 1.0000x reference)
"""Trainium2 Bass kernel for the non-local attention block (nn_CPP_80676665688885).

Sharding: pure data-parallel over batch — 1 sample per NeuronCore (B=8, 8 cores).
BatchNorm batch-statistics are combined with a tiny (2 KB) AllGather.

fp32 matmuls on TRN2 run in LOW_HIGH mode (2 passes, ~2.5 cyc/col) — ~5x the
cost of bf16. So every large matmul here is decomposed into bf16 passes:
  exact-ish (error ~2^-16): A@B = A_hi@B_hi + A_hi@B_lo + A_lo@B_hi
  where X_hi = bf16(X), X_lo = bf16(X - X_hi); fp32 accumulation in PSUM.
exp(fT) is written directly as bf16: its quantization acts as a correlated
perturbation of softmax logits (numerator and denominator use the same
values), so the final error stays ~1e-4 relative.

Per-core algorithm (sample x: (C=256, N=4096), N = 64x64 spatial):
  theta = Wt@x + bt  (split hi/lo)     phi,g = maxpool2(conv)  (phi split, g
  transposed then split)
  fT    = phi^T @ theta  3 bf16 passes; exp on ScalarE -> expf bf16
  y     = gT^T @ expf    2 bf16 passes (gT hi/lo), accumulated over m-chunks
  s[n]  = ones^T @ expf  1 bf16 pass,  accumulated over m-chunks
  y_n   = y * (1/s)  (reciprocal exactly on (128,x) layout via DRAM bounce)
  wy    = Ww @ y_n   (native fp32; bias bw dropped — cancels in BatchNorm)
  S1,S2 per channel -> AllGather over 8 cores -> local sum
  z     = (wy - mean)*rsqrt(var+eps)*gamma + beta + x ; out = max_n z
"""

import numpy as np
from contextlib import ExitStack

import concourse.bass as bass
import concourse.bacc as bacc
import concourse.tile as tile
from concourse import mybir
from concourse.bass_utils import run_bass_kernel_spmd

F32 = mybir.dt.float32
BF16 = mybir.dt.bfloat16
AF = mybir.ActivationFunctionType
ALU = mybir.AluOpType
AX = mybir.AxisListType

B = 8
C = 256
CI = 128
N = 4096          # 64*64
M = 1024          # 32*32 after 2x2 maxpool
NT = 512          # n-tile (PSUM bank width in fp32)
NTILES = N // NT  # 8
MCH = M // 128    # 8 m-chunks
CCH = C // 128    # 2 channel chunks
EPS = 1e-5
INV_CNT = 1.0 / (B * N)

_CACHE = {}


def _build():
    nc = bacc.Bacc("TRN2", num_devices=B)

    x_d = nc.declare_dram_parameter("x", [C, N], F32, False)
    # hi/lo bf16-split projection weights, pre-transposed host-side
    w_hi_d = {}
    w_lo_d = {}
    for nm in ("t", "p", "g"):
        w_hi_d[nm] = nc.declare_dram_parameter(f"W{nm}Thi", [C, CI], BF16, False)
        w_lo_d[nm] = nc.declare_dram_parameter(f"W{nm}Tlo", [C, CI], BF16, False)
    wwT_hi_d = nc.declare_dram_parameter("WwThi", [CI, C], BF16, False)
    wwT_lo_d = nc.declare_dram_parameter("WwTlo", [CI, C], BF16, False)
    bt_d = nc.declare_dram_parameter("bt", [CI, 1], F32, False)
    bp_d = nc.declare_dram_parameter("bp", [CI, 1], F32, False)
    bg_d = nc.declare_dram_parameter("bg", [CI, 1], F32, False)
    gamma_d = nc.declare_dram_parameter("gamma", [128, CCH], F32, False)
    beta_d = nc.declare_dram_parameter("beta", [128, CCH], F32, False)
    out_d = nc.declare_dram_parameter("out", [CCH, 128], F32, True)

    ident_d = nc.inline_tensor(np.eye(128, dtype=np.float32), name="ident")

    # DRAM bounce buffers
    s_dram = nc.dram_tensor("s_bounce", [1, N], F32)
    r_hi_dram = nc.dram_tensor("r_hi_bounce", [1, N], BF16)
    r_lo_dram = nc.dram_tensor("r_lo_bounce", [1, N], BF16)
    warm_in = nc.dram_tensor("warm_in", [1, 8], F32)
    warm_out = nc.dram_tensor("warm_out", [1, 8], F32, addr_space="Shared")
    stats_in = nc.dram_tensor("stats_in", [128, 2 * CCH], F32)
    stats_out = nc.dram_tensor("stats_out", [128, 2 * CCH], F32,
                               addr_space="Shared")

    with ExitStack() as ctx:
        tc = ctx.enter_context(tile.TileContext(nc))
        consts = ctx.enter_context(tc.tile_pool(name="consts", bufs=1))
        persist = ctx.enter_context(tc.tile_pool(name="persist", bufs=1))
        scratch = ctx.enter_context(tc.tile_pool(name="scratch", bufs=2))
        efp = ctx.enter_context(tc.tile_pool(name="efp", bufs=5))
        small = ctx.enter_context(tc.tile_pool(name="small", bufs=4))
        ps_ft = ctx.enter_context(tc.tile_pool(name="ps_ft", bufs=2, space="PSUM"))
        ps_y = ctx.enter_context(tc.tile_pool(name="ps_y", bufs=2, space="PSUM"))
        ps_s = ctx.enter_context(tc.tile_pool(name="ps_s", bufs=1, space="PSUM"))
        ps_rb = ctx.enter_context(tc.tile_pool(name="ps_rb", bufs=1, space="PSUM"))
        ps_cv = ctx.enter_context(tc.tile_pool(name="ps_cv", bufs=2, space="PSUM"))

        # ---- constants / weights into SBUF ----
        ident = consts.tile([128, 128], F32)
        nc.sync.dma_start(out=ident, in_=ident_d[:, :])
        ones_k = consts.tile([128, 1], BF16)
        nc.vector.memset(ones_k, 1.0)
        ones_p = consts.tile([1, 128], BF16)
        nc.vector.memset(ones_p, 1.0)
        eps_sb = consts.tile([128, 1], F32)
        nc.vector.memset(eps_sb, EPS)

        w_hi = {}
        w_lo = {}
        for nm in ("t", "p", "g"):
            w_hi[nm] = consts.tile([128, CCH, CI], BF16, name=f"w_hi_{nm}")
            w_lo[nm] = consts.tile([128, CCH, CI], BF16, name=f"w_lo_{nm}")
            for ch in range(CCH):
                cs = slice(ch * 128, (ch + 1) * 128)
                nc.sync.dma_start(out=w_hi[nm][:, ch, :], in_=w_hi_d[nm][cs, :])
                nc.sync.dma_start(out=w_lo[nm][:, ch, :], in_=w_lo_d[nm][cs, :])
        ww_hi = consts.tile([128, CCH, 128], BF16)
        ww_lo = consts.tile([128, CCH, 128], BF16)
        for ch in range(CCH):
            nc.sync.dma_start(out=ww_hi[:, ch, :], in_=wwT_hi_d[:, ch * 128:(ch + 1) * 128])
            nc.sync.dma_start(out=ww_lo[:, ch, :], in_=wwT_lo_d[:, ch * 128:(ch + 1) * 128])
        bt_sb = consts.tile([128, 1], F32)
        bp_sb = consts.tile([128, 1], F32)
        bg_sb = consts.tile([128, 1], F32)
        nc.sync.dma_start(out=bt_sb, in_=bt_d[:, :])
        nc.sync.dma_start(out=bp_sb, in_=bp_d[:, :])
        nc.sync.dma_start(out=bg_sb, in_=bg_d[:, :])
        gamma_sb = consts.tile([128, CCH], F32)
        beta_sb = consts.tile([128, CCH], F32)
        nc.sync.dma_start(out=gamma_sb, in_=gamma_d[:, :])
        nc.sync.dma_start(out=beta_sb, in_=beta_d[:, :])

        # warm up the collective path early (overlaps with compute)
        warm_sb = small.tile([1, 8], F32, tag="warm")
        nc.vector.memset(warm_sb, 1.0)
        nc.sync.dma_start(out=warm_in[:, :], in_=warm_sb)
        nc.gpsimd.collective_compute(
            "AllReduce", ALU.add, replica_groups=[list(range(B))],
            ins=[warm_in[:, :]], outs=[warm_out[:, :]])

        # ---- x into SBUF, split hi/lo ----
        x_sb = [persist.tile([128, N], F32, tag=f"x{ch}", name=f"x_sb{ch}")
                for ch in range(CCH)]
        x_hi = [persist.tile([128, N], BF16, tag=f"xh{ch}", name=f"x_hi{ch}")
                for ch in range(CCH)]
        x_lo = [persist.tile([128, N], BF16, tag=f"xl{ch}", name=f"x_lo{ch}")
                for ch in range(CCH)]
        for ch in range(CCH):
            nc.sync.dma_start(out=x_sb[ch], in_=x_d[ch * 128:(ch + 1) * 128, :])
            nc.scalar.copy(out=x_hi[ch], in_=x_sb[ch])
            nc.vector.tensor_tensor(out=x_lo[ch], in0=x_sb[ch], in1=x_hi[ch],
                                    op=ALU.subtract)

        # ---- projections (3-term bf16 conv) ----
        # theta: kept as hi/lo bf16 tiles; phi/g: fp32 for pooling
        th_hi = persist.tile([128, N], BF16, tag="thh")
        th_lo = persist.tile([128, N], BF16, tag="thl")
        phi_full = scratch.tile([128, N], F32, tag="s4")
        g_full = scratch.tile([128, N], F32, tag="s4")

        def conv_mms(ps, nm, sl):
            terms = ((w_hi[nm], x_hi), (w_hi[nm], x_lo), (w_lo[nm], x_hi))
            nterm = len(terms) * CCH
            k = 0
            for ch in range(CCH):
                for lhs, rhs in terms:
                    nc.tensor.matmul(ps, lhsT=lhs[:, ch, :], rhs=rhs[ch][:, sl],
                                     start=(k == 0), stop=(k == nterm - 1))
                    k += 1

        for it in range(NTILES):
            sl = slice(it * NT, (it + 1) * NT)
            ps = ps_cv.tile([128, NT], F32, tag="cv")
            conv_mms(ps, "t", sl)
            # theta + bias, split hi/lo (hi on ScalarE, lo on VectorE)
            nc.scalar.activation(out=th_hi[:, sl], in_=ps, func=AF.Identity,
                                 bias=bt_sb, scale=1.0)
            nc.vector.scalar_tensor_tensor(out=th_lo[:, sl], in0=ps, scalar=bt_sb,
                                           in1=th_hi[:, sl], op0=ALU.add,
                                           op1=ALU.subtract)
        for dst, nm, b_sb in ((phi_full, "p", bp_sb), (g_full, "g", bg_sb)):
            for it in range(NTILES):
                sl = slice(it * NT, (it + 1) * NT)
                ps = ps_cv.tile([128, NT], F32, tag="cv")
                conv_mms(ps, nm, sl)
                nc.vector.tensor_scalar_add(out=dst[:, sl], in0=ps, scalar1=b_sb)

        # ---- 2x2 maxpool on phi and g ----
        phi_pool = persist.tile([128, M], F32, tag="phip")
        g_pool = persist.tile([128, M], F32, tag="gp")
        pp1 = scratch.tile([128, 64 * 32], F32, tag="pool1")
        gp1 = scratch.tile([128, 64 * 32], F32, tag="pool1")
        for src, mid, dst in ((phi_full, pp1, phi_pool), (g_full, gp1, g_pool)):
            sr = src.rearrange("p (h wp t) -> p h wp t", h=64, wp=32, t=2)
            nc.vector.tensor_tensor(
                out=mid.rearrange("p (h wp) -> p h wp", h=64),
                in0=sr[:, :, :, 0], in1=sr[:, :, :, 1], op=ALU.max)
            mr = mid.rearrange("p (hp s wp) -> p hp s wp", hp=32, s=2, wp=32)
            nc.vector.tensor_tensor(
                out=dst.rearrange("p (hp wp) -> p hp wp", hp=32),
                in0=mr[:, :, 0, :], in1=mr[:, :, 1, :], op=ALU.max)

        # phi hi/lo split
        phi_hi = persist.tile([128, M], BF16, tag="phih")
        phi_lo = persist.tile([128, M], BF16, tag="phil")
        nc.scalar.copy(out=phi_hi, in_=phi_pool)
        nc.vector.tensor_tensor(out=phi_lo, in0=phi_pool, in1=phi_hi,
                                op=ALU.subtract)

        # ---- transpose g_pool (CI, M) -> gT chunks (m=128, CI), split hi/lo ----
        gT32 = persist.tile([128, MCH, CI], F32, tag="gT32")
        gT_hi = persist.tile([128, MCH, CI], BF16, tag="gTh")
        gT_lo = persist.tile([128, MCH, CI], BF16, tag="gTl")
        for mc in range(MCH):
            tp = ps_cv.tile([128, 128], F32, tag="cv")
            nc.tensor.transpose(tp, g_pool[:, mc * 128:(mc + 1) * 128], ident)
            nc.scalar.copy(out=gT32[:, mc, :], in_=tp)
            nc.scalar.copy(out=gT_hi[:, mc, :], in_=gT32[:, mc, :])
            nc.vector.tensor_tensor(out=gT_lo[:, mc, :], in0=gT32[:, mc, :],
                                    in1=gT_hi[:, mc, :], op=ALU.subtract)

        # ---- attention + normalization + W-conv, per n-tile ----
        y_hi = persist.tile([128, N], BF16, tag="ynh")
        y_lo = persist.tile([128, N], BF16, tag="ynl")
        wy = [scratch.tile([128, N], F32, tag="s4", name=f"wy{ch}")
              for ch in range(CCH)]
        rT = persist.tile([128, NTILES * (NT // 128)], F32, tag="rT")
        s1p = persist.tile([128, CCH, NTILES], F32, tag="s1p")
        s2p = persist.tile([128, CCH, NTILES], F32, tag="s2p")

        for it in range(NTILES):
            sl = slice(it * NT, (it + 1) * NT)
            yps = ps_y.tile([128, NT], F32, tag="yps")
            sps = ps_s.tile([1, NT], F32, tag="sps")
            for mc in range(MCH):
                ms = slice(mc * 128, (mc + 1) * 128)
                fps = ps_ft.tile([128, NT], F32, tag="ft")
                nc.tensor.matmul(fps, lhsT=phi_hi[:, ms], rhs=th_hi[:, sl],
                                 start=True, stop=False)
                nc.tensor.matmul(fps, lhsT=phi_hi[:, ms], rhs=th_lo[:, sl],
                                 start=False, stop=False)
                nc.tensor.matmul(fps, lhsT=phi_lo[:, ms], rhs=th_hi[:, sl],
                                 start=False, stop=True)
                ef = efp.tile([128, NT], BF16, tag="ef")
                nc.scalar.activation(out=ef, in_=fps, func=AF.Exp)
                nc.tensor.matmul(yps, lhsT=gT_hi[:, mc, :], rhs=ef,
                                 start=(mc == 0), stop=False)
                nc.tensor.matmul(yps, lhsT=gT_lo[:, mc, :], rhs=ef,
                                 start=False, stop=(mc == MCH - 1))
                nc.tensor.matmul(sps, lhsT=ones_k, rhs=ef,
                                 start=(mc == 0), stop=(mc == MCH - 1))

            # s -> SBUF, bounce via DRAM into (128, NT/128) layout, recip, back
            s_sb = small.tile([1, NT], F32, tag="s1d")
            nc.scalar.copy(out=s_sb, in_=sps)
            nc.sync.dma_start(out=s_dram[:, sl], in_=s_sb)
            f4 = NT // 128
            sl4 = slice(it * f4, (it + 1) * f4)
            sT_t = small.tile([128, f4], F32, tag="sT")
            nc.sync.dma_start(out=sT_t, in_=s_dram[0, sl].rearrange("(p f) -> p f", p=128))
            nc.vector.reciprocal(out=rT[:, sl4], in_=sT_t)
            rT_hi = small.tile([128, f4], BF16, tag="rTh")
            rT_lo = small.tile([128, f4], BF16, tag="rTl")
            nc.vector.tensor_copy(out=rT_hi, in_=rT[:, sl4])
            nc.vector.tensor_tensor(out=rT_lo, in0=rT[:, sl4], in1=rT_hi, op=ALU.subtract)
            nc.sync.dma_start(out=r_hi_dram[0, sl].rearrange("(p f) -> p f", p=128), in_=rT_hi)
            nc.sync.dma_start(out=r_lo_dram[0, sl].rearrange("(p f) -> p f", p=128), in_=rT_lo)
            r_hi_sb = small.tile([1, NT], BF16, tag="r1dh")
            r_lo_sb = small.tile([1, NT], BF16, tag="r1dl")
            nc.sync.dma_start(out=r_hi_sb, in_=r_hi_dram[:, sl])
            nc.sync.dma_start(out=r_lo_sb, in_=r_lo_dram[:, sl])

            # broadcast r across partitions via two K=1 bf16 matmuls
            rbps = ps_rb.tile([128, NT], F32, tag="rb")
            nc.tensor.matmul(rbps, lhsT=ones_p, rhs=r_hi_sb, start=True, stop=False)
            nc.tensor.matmul(rbps, lhsT=ones_p, rhs=r_lo_sb, start=False, stop=True)
            rb_sb = small.tile([128, NT], F32, tag="rb_sb")
            nc.scalar.copy(out=rb_sb, in_=rbps)
            nc.vector.scalar_tensor_tensor(
                out=y_hi[:, sl], in0=yps, scalar=1.0, in1=rb_sb,
                op0=ALU.mult, op1=ALU.mult)
            # y_lo = y - y_hi = (yps*rb) - y_hi
            nc.vector.scalar_tensor_tensor(
                out=y_lo[:, sl], in0=yps, scalar=1.0, in1=rb_sb,
                op0=ALU.mult, op1=ALU.mult)
            nc.vector.tensor_tensor(out=y_lo[:, sl], in0=y_lo[:, sl],
                                    in1=y_hi[:, sl], op=ALU.subtract)

            # W conv (3-term bf16); accumulate BN partial stats
            for ch in range(CCH):
                wps = ps_cv.tile([128, NT], F32, tag="cv")
                nc.tensor.matmul(wps, lhsT=ww_hi[:, ch, :], rhs=y_hi[:, sl],
                                 start=True, stop=False)
                nc.tensor.matmul(wps, lhsT=ww_hi[:, ch, :], rhs=y_lo[:, sl],
                                 start=False, stop=False)
                nc.tensor.matmul(wps, lhsT=ww_lo[:, ch, :], rhs=y_hi[:, sl],
                                 start=False, stop=True)
                nc.vector.tensor_scalar(
                    out=wy[ch][:, sl], in0=wps, scalar1=0.0, scalar2=None,
                    op0=ALU.add, op1=ALU.add, accum_out=s1p[:, ch, it:it + 1])
                sqt = efp.tile([128, NT], BF16, tag="sqtrash")
                nc.scalar.activation(
                    out=sqt, in_=wy[ch][:, sl], func=AF.Square,
                    accum_out=s2p[:, ch, it:it + 1])

        # ---- combine partials, AllGather, local sum, finalize ----
        stats_sb = small.tile([128, 2 * CCH], F32, tag="stats")
        for ch in range(CCH):
            nc.vector.tensor_reduce(out=stats_sb[:, 2 * ch:2 * ch + 1],
                                    in_=s1p[:, ch, :], axis=AX.X, op=ALU.add)
            nc.vector.tensor_reduce(out=stats_sb[:, 2 * ch + 1:2 * ch + 2],
                                    in_=s2p[:, ch, :], axis=AX.X, op=ALU.add)
        nc.sync.dma_start(out=stats_in[:, :], in_=stats_sb)
        nc.gpsimd.collective_compute(
            "AllReduce", ALU.add, replica_groups=[list(range(B))],
            ins=[stats_in[:, :]], outs=[stats_out[:, :]])
        stats_g = small.tile([128, 2 * CCH], F32, tag="statsg")
        nc.sync.dma_start(out=stats_g, in_=stats_out[:, :])

        out_sb = small.tile([128, CCH], F32, tag="outsb")
        for ch in range(CCH):
            mean = small.tile([128, 1], F32, tag="fin")
            e2 = small.tile([128, 1], F32, tag="fin")
            m2 = small.tile([128, 1], F32, tag="fin")
            var = small.tile([128, 1], F32, tag="fin")
            nc.vector.tensor_scalar_mul(out=mean, in0=stats_g[:, 2 * ch:2 * ch + 1],
                                        scalar1=INV_CNT)
            nc.vector.tensor_scalar_mul(out=e2, in0=stats_g[:, 2 * ch + 1:2 * ch + 2],
                                        scalar1=INV_CNT)
            nc.scalar.square(out=m2, in_=mean)
            nc.vector.tensor_tensor(out=var, in0=e2, in1=m2, op=ALU.subtract)
            sd = small.tile([128, 1], F32, tag="fin")
            nc.scalar.activation(out=sd, in_=var, func=AF.Sqrt, bias=eps_sb, scale=1.0)
            inv = small.tile([128, 1], F32, tag="fin")
            nc.vector.reciprocal(out=inv, in_=sd)
            scale = small.tile([128, 1], F32, tag="fin")
            nc.vector.tensor_tensor(out=scale, in0=inv, in1=gamma_sb[:, ch:ch + 1],
                                    op=ALU.mult)
            negshift = small.tile([128, 1], F32, tag="fin")
            nc.vector.scalar_tensor_tensor(
                out=negshift, in0=mean, scalar=scale, in1=beta_sb[:, ch:ch + 1],
                op0=ALU.mult, op1=ALU.subtract)
            # z' = wy*scale + x (in place over wy); out = max_n z' - negshift
            nc.vector.scalar_tensor_tensor(
                out=wy[ch][:, :], in0=wy[ch][:, :], scalar=scale, in1=x_sb[ch],
                op0=ALU.mult, op1=ALU.add)
            mx = small.tile([128, 1], F32, tag="fin")
            nc.vector.tensor_reduce(out=mx, in_=wy[ch][:, :], axis=AX.X, op=ALU.max)
            nc.vector.tensor_tensor(out=out_sb[:, ch:ch + 1], in0=mx, in1=negshift,
                                    op=ALU.subtract)
        for ch in range(CCH):
            nc.sync.dma_start(out=out_d[ch, :].rearrange("(p one) -> p one", one=1),
                              in_=out_sb[:, ch:ch + 1])

    nc.compile()
    return nc


_LAST = {}


def kernel(**inputs):
    x = np.ascontiguousarray(inputs["x"], dtype=np.float32)      # (8, 256, 64, 64)
    Wg = np.asarray(inputs["Wg"], dtype=np.float32)
    bg = np.asarray(inputs["bg"], dtype=np.float32)
    Wt = np.asarray(inputs["Wt"], dtype=np.float32)
    bt = np.asarray(inputs["bt"], dtype=np.float32)
    Wp = np.asarray(inputs["Wp"], dtype=np.float32)
    bp = np.asarray(inputs["bp"], dtype=np.float32)
    Ww = np.asarray(inputs["Ww"], dtype=np.float32)
    gamma = np.asarray(inputs["gamma"], dtype=np.float32)
    beta = np.asarray(inputs["beta"], dtype=np.float32)

    if "nc" not in _CACHE:
        _CACHE["nc"] = _build()
    nc = _CACHE["nc"]

    try:
        import ml_dtypes
        bf = ml_dtypes.bfloat16
    except ImportError:
        import jax.numpy as jnp
        bf = jnp.bfloat16

    def split(w):
        hi = np.ascontiguousarray(w.astype(bf))
        lo = np.ascontiguousarray((w - hi.astype(np.float32)).astype(bf))
        return hi, lo

    WtThi, WtTlo = split(np.ascontiguousarray(Wt.T))
    WpThi, WpTlo = split(np.ascontiguousarray(Wp.T))
    WgThi, WgTlo = split(np.ascontiguousarray(Wg.T))
    WwThi, WwTlo = split(np.ascontiguousarray(Ww.T))

    shared = {
        "WtThi": WtThi, "WtTlo": WtTlo,
        "WpThi": WpThi, "WpTlo": WpTlo,
        "WgThi": WgThi, "WgTlo": WgTlo,
        "WwThi": WwThi, "WwTlo": WwTlo,
        "bt": np.ascontiguousarray(bt.reshape(CI, 1)),
        "bp": np.ascontiguousarray(bp.reshape(CI, 1)),
        "bg": np.ascontiguousarray(bg.reshape(CI, 1)),
        "gamma": np.ascontiguousarray(gamma.reshape(CCH, 128).T),
        "beta": np.ascontiguousarray(beta.reshape(CCH, 128).T),
    }
    in_maps = [dict(shared, x=np.ascontiguousarray(x[b].reshape(C, N)))
               for b in range(B)]
    import os
    trace = bool(int(os.environ.get("KERNEL_TRACE", "0")))
    res = run_bass_kernel_spmd(nc, in_maps, core_ids=list(range(B)), trace=trace)
    _LAST["res"] = res
    out = np.stack([np.asarray(res.results[b]["out"]).reshape(C) for b in range(B)])
    return out.reshape(B, C, 1, 1).astype(np.float32)


if __name__ == "__main__":
    pass



# revision 13
# speedup vs baseline: 1.9442x; 1.9442x over previous
"""Trainium2 Bass kernel for the non-local attention block (nn_CPP_80676665688885).

Sharding: pure data-parallel over batch - 1 sample per NeuronCore (B=8, 8 cores).
BatchNorm batch-statistics are combined with a tiny (2 KB) AllReduce.

Precision scheme (validated vs reference in numpy, rel-err ~8e-3 < 2e-2):
  - All matmuls single-pass: stationary (weight) operands bf16 (fast weight
    load), moving operands float32r (1 cyc/col when free >= 256) or bf16.
  - exp(fT) emitted as bf16 (correlated perturbation cancels in softmax).
  - Softmax denominators computed by an all-ones (128,128) bf16 matmul so the
    PSUM result is already broadcast across partitions; 1/s via
    reciprocal_approx_fast (~18 bits), y_n = yps * r elementwise.
  - Biases bp, bg, bw cancel mathematically (bp: per-n constant in softmax
    logits; bg: additive constant absorbed by BatchNorm mean; bw: same).
    Only bt survives and is folded into the theta PSUM->SBUF bias-add.

Per-core algorithm (sample x: (C=256, N=4096), N = 64x64 spatial):
  theta = Wt@x + bt  (f32); phi,g = maxpool2(conv) pooled straight out of
  PSUM (DVE for phi, Pool engine for g), stored bf16
  gT    = transpose(g_pool) via PE (bf16)
  per n-tile (512): fT = phi_mc^T @ theta; ef = exp(fT) bf16
     y += gT_mc^T @ ef ; s += ones^T @ ef (s lands broadcast on 128 parts)
     y_n = y * approx(1/s); wy_ch = Ww_ch @ y_n -> bf16 + S1/S2 accumulation
  stats AllReduce (warmed by dummy AllReduces during compute)
  z = scale*wy + x (bf16), out = max_n z + shift
"""

import numpy as np
from contextlib import ExitStack

import concourse.bass as bass
import concourse.bacc as bacc
import concourse.tile as tile
from concourse import mybir
from concourse.bass_utils import run_bass_kernel_spmd

F32 = mybir.dt.float32
F32R = mybir.dt.float32r
BF16 = mybir.dt.bfloat16
AF = mybir.ActivationFunctionType
ALU = mybir.AluOpType
AX = mybir.AxisListType

B = 8
C = 256
CI = 128
N = 4096          # 64*64
M = 1024          # 32*32 after 2x2 maxpool
NT = 512          # n-tile (PSUM bank width in fp32)
NTILES = N // NT  # 8
MCH = M // 128    # 8 m-chunks
CCH = C // 128    # 2 channel chunks
EPS = 1e-5
INV_CNT = 1.0 / (B * N)

_CACHE = {}
_LAST = {}


def _build():
    import ml_dtypes

    nc = bacc.Bacc("TRN2", num_devices=B)

    x_d = nc.declare_dram_parameter("x", [C, N], F32R, False)
    wT_d = {nm: nc.declare_dram_parameter(f"W{nm}T", [C, CI], F32R, False)
            for nm in ("t", "p", "g")}
    wwT_d = nc.declare_dram_parameter("WwT", [CI, C], F32R, False)
    bt_d = nc.declare_dram_parameter("bt", [CI, 1], F32, False)
    gamma_d = nc.declare_dram_parameter("gamma", [128, CCH], F32, False)
    beta_d = nc.declare_dram_parameter("beta", [128, CCH], F32, False)
    out_d = nc.declare_dram_parameter("out", [CCH, 128], F32, True)

    ident_bf_d = nc.inline_tensor(
        np.eye(128).astype(ml_dtypes.bfloat16), name="identbf")

    warm_in = [nc.dram_tensor(f"warm_in{i}", [1, 8], F32) for i in range(3)]
    warm_out = [nc.dram_tensor(f"warm_out{i}", [1, 8], F32, addr_space="Shared")
                for i in range(3)]
    stats_in = nc.dram_tensor("stats_in", [128, 2 * CCH], F32)
    stats_out = nc.dram_tensor("stats_out", [128, 2 * CCH], F32,
                               addr_space="Shared")

    with ExitStack() as ctx:
        tc = ctx.enter_context(tile.TileContext(nc))
        consts = ctx.enter_context(tc.tile_pool(name="consts", bufs=1))
        persist = ctx.enter_context(tc.tile_pool(name="persist", bufs=1))
        mids = ctx.enter_context(tc.tile_pool(name="mids", bufs=2))
        efp = ctx.enter_context(tc.tile_pool(name="efp", bufs=4))
        nrm = ctx.enter_context(tc.tile_pool(name="nrm", bufs=2))
        small = ctx.enter_context(tc.tile_pool(name="small", bufs=4))
        ps_cv = ctx.enter_context(tc.tile_pool(name="ps_cv", bufs=2, space="PSUM"))
        ps_ft = ctx.enter_context(tc.tile_pool(name="ps_ft", bufs=2, space="PSUM"))
        ps_y = ctx.enter_context(tc.tile_pool(name="ps_y", bufs=2, space="PSUM"))
        ps_s = ctx.enter_context(tc.tile_pool(name="ps_s", bufs=2, space="PSUM"))

        # ---- warmup collective first (overlaps the x DMA head) ----
        warm_sb = small.tile([1, 8], F32, tag="warm")
        nc.vector.memset(warm_sb, 1.0)
        nc.gpsimd.dma_start(out=warm_in[0][:, :], in_=warm_sb)
        nc.gpsimd.collective_compute(
            "AllReduce", ALU.add, replica_groups=[list(range(B))],
            ins=[warm_in[0][:, :]], outs=[warm_out[0][:, :]])

        # ---- weights + constants (sync queue) ----
        ident_bf = consts.tile([128, 128], BF16)
        nc.sync.dma_start(out=ident_bf, in_=ident_bf_d[:, :])
        ones_bf = consts.tile([128, 128], BF16)
        nc.vector.memset(ones_bf, 1.0)
        w_sb = {}
        for nm in ("t", "p", "g"):
            w_sb[nm] = consts.tile([128, CCH, CI], F32R, name=f"w_{nm}")
            for ch in range(CCH):
                nc.sync.dma_start(out=w_sb[nm][:, ch, :],
                                  in_=wT_d[nm][ch * 128:(ch + 1) * 128, :])
        ww_sb = consts.tile([128, CCH, 128], F32R)
        for ch in range(CCH):
            nc.sync.dma_start(out=ww_sb[:, ch, :],
                              in_=wwT_d[:, ch * 128:(ch + 1) * 128])
        bt_sb = consts.tile([128, 1], F32)
        nc.sync.dma_start(out=bt_sb, in_=bt_d[:, :])
        gamma_sb = consts.tile([128, CCH], F32)
        beta_sb = consts.tile([128, CCH], F32)
        nc.sync.dma_start(out=gamma_sb, in_=gamma_d[:, :])
        nc.sync.dma_start(out=beta_sb, in_=beta_d[:, :])
        eps_sb = consts.tile([128, 1], F32)
        nc.vector.memset(eps_sb, EPS)

        # ---- x: 16 tile-sized DMAs split over the two HWDGE queues ----
        x_sb = [persist.tile([128, N], F32R, tag=f"x{ch}", name=f"x{ch}")
                for ch in range(CCH)]
        for it in range(NTILES):
            sl = slice(it * NT, (it + 1) * NT)
            nc.sync.dma_start(out=x_sb[0][:, sl], in_=x_d[0:128, sl])
            nc.scalar.dma_start(out=x_sb[1][:, sl], in_=x_d[128:256, sl])

        # ---- stage B: projections + fused pooling ----
        theta = persist.tile([128, N], F32R, tag="theta")
        phi_pool = persist.tile([128, M], F32R, tag="phip")
        g_pool = persist.tile([128, M], BF16, tag="gp")

        def conv2(ps, nm, sl):
            nc.tensor.matmul(ps, lhsT=w_sb[nm][:, 0, :],
                             rhs=x_sb[0][:, sl],
                             start=True, stop=False)
            nc.tensor.matmul(ps, lhsT=w_sb[nm][:, 1, :],
                             rhs=x_sb[1][:, sl],
                             start=False, stop=True)

        for it in range(NTILES):
            sl = slice(it * NT, (it + 1) * NT)
            msl = slice(it * 128, (it + 1) * 128)
            ps = ps_cv.tile([128, NT], F32, tag="cv")
            conv2(ps, "t", sl)
            nc.scalar.activation(out=theta[:, sl], in_=ps, func=AF.Identity,
                                 bias=bt_sb, scale=1.0)
            # 2x2 maxpool fused out of PSUM: one XY-reduce over the
            # (row-pair, col-pair) innermost axes per projection
            for nm, dst in (("p", phi_pool), ("g", g_pool)):
                ps2 = ps_cv.tile([128, NT], F32, tag="cv")
                conv2(ps2, nm, sl)
                pr = ps2.rearrange("p (hp s wp t) -> p hp wp s t",
                                   hp=4, s=2, wp=32, t=2)
                nc.vector.tensor_reduce(
                    out=dst[:, msl].rearrange("p (hp wp) -> p hp wp", hp=4),
                    in_=pr, axis=AX.XY, op=ALU.max)

        # ---- transpose g_pool -> gT (bf16, via PE; one PSUM bank) ----
        gT = persist.tile([128, MCH, CI], BF16, tag="gT")
        tp = ps_cv.tile([128, MCH, 128], BF16, tag="cv")
        for mc in range(MCH):
            nc.tensor.transpose(tp[:, mc, :],
                                g_pool[:, mc * 128:(mc + 1) * 128], ident_bf)
        nc.vector.tensor_copy(out=gT[:, :, :], in_=tp[:, :, :])

        # ---- attention + normalization + W-conv, per n-tile ----
        wy = [persist.tile([128, N], BF16, tag=f"wy{ch}", name=f"wy{ch}")
              for ch in range(CCH)]
        x_bf = [persist.tile([128, N], BF16, tag=f"xb{ch}", name=f"xb{ch}")
                for ch in range(CCH)]
        s1p = persist.tile([128, CCH, NTILES], F32, tag="s1p")
        s2p = persist.tile([128, CCH, NTILES], F32, tag="s2p")
        sq_trash = persist.tile([128, NT], BF16, tag="sqt")

        for it in range(NTILES):
            sl = slice(it * NT, (it + 1) * NT)
            yps = ps_y.tile([128, NT], F32, tag="y")
            sps = ps_s.tile([128, NT], F32, tag="s")
            for mc in range(MCH):
                fps = ps_ft.tile([128, NT], F32, tag="ft")
                nc.tensor.matmul(fps,
                                 lhsT=phi_pool[:, mc * 128:(mc + 1) * 128],
                                 rhs=theta[:, sl],
                                 start=True, stop=True)
                ef = efp.tile([128, NT], BF16, tag="ef")
                nc.scalar.activation(out=ef, in_=fps, func=AF.Exp)
                nc.tensor.matmul(yps, lhsT=gT[:, mc, :], rhs=ef,
                                 start=(mc == 0), stop=(mc == MCH - 1))
                nc.tensor.matmul(sps, lhsT=ones_bf, rhs=ef,
                                 start=(mc == 0), stop=(mc == MCH - 1))
            rb = nrm.tile([128, NT], F32, tag="rb")
            nc.vector.reciprocal_approx_fast(out=rb, in_=sps)
            y_n = nrm.tile([128, NT], F32R, tag="yn")
            nc.vector.tensor_tensor(out=y_n, in0=yps, in1=rb, op=ALU.mult)
            for ch in range(CCH):
                wps = ps_cv.tile([128, NT], F32, tag="cv")
                nc.tensor.matmul(wps, lhsT=ww_sb[:, ch, :],
                                 rhs=y_n, start=True, stop=True)
                nc.vector.tensor_scalar(
                    out=wy[ch][:, sl], in0=wps, scalar1=0.0, scalar2=None,
                    op0=ALU.add, op1=ALU.add,
                    accum_out=s1p[:, ch, it:it + 1])
                nc.vector.scalar_tensor_tensor(
                    out=sq_trash, in0=wy[ch][:, sl], scalar=1.0,
                    in1=wy[ch][:, sl], op0=ALU.mult, op1=ALU.mult,
                    accum_out=s2p[:, ch, it:it + 1])
            for ch in range(CCH):
                nc.vector.tensor_copy(out=x_bf[ch][:, sl], in_=x_sb[ch][:, sl])
            if it in (2, 5):
                i = 1 if it == 2 else 2
                nc.gpsimd.dma_start(out=warm_in[i][:, :], in_=warm_sb)
                nc.gpsimd.collective_compute(
                    "AllReduce", ALU.add, replica_groups=[list(range(B))],
                    ins=[warm_in[i][:, :]], outs=[warm_out[i][:, :]])

        # ---- stats, AllReduce, finalize ----
        stats_sb = small.tile([128, 2 * CCH], F32, tag="stats")
        for ch in range(CCH):
            nc.vector.tensor_reduce(out=stats_sb[:, 2 * ch:2 * ch + 1],
                                    in_=s1p[:, ch, :], axis=AX.X, op=ALU.add)
            nc.vector.tensor_reduce(out=stats_sb[:, 2 * ch + 1:2 * ch + 2],
                                    in_=s2p[:, ch, :], axis=AX.X, op=ALU.add)
        nc.gpsimd.dma_start(out=stats_in[:, :], in_=stats_sb)
        nc.gpsimd.collective_compute(
            "AllReduce", ALU.add, replica_groups=[list(range(B))],
            ins=[stats_in[:, :]], outs=[stats_out[:, :]])
        stats_g = small.tile([128, 2 * CCH], F32, tag="statsg")
        nc.sync.dma_start(out=stats_g, in_=stats_out[:, :])

        zt = persist.tile([128, N], BF16, tag="zt")
        out_sb = small.tile([128, CCH], F32, tag="outsb")
        for ch in range(CCH):
            mean = small.tile([128, 1], F32, tag="fin")
            e2 = small.tile([128, 1], F32, tag="fin")
            m2 = small.tile([128, 1], F32, tag="fin")
            var = small.tile([128, 1], F32, tag="fin")
            nc.vector.tensor_scalar_mul(out=mean,
                                        in0=stats_g[:, 2 * ch:2 * ch + 1],
                                        scalar1=INV_CNT)
            nc.vector.tensor_scalar_mul(out=e2,
                                        in0=stats_g[:, 2 * ch + 1:2 * ch + 2],
                                        scalar1=INV_CNT)
            nc.scalar.square(out=m2, in_=mean)
            nc.vector.tensor_tensor(out=var, in0=e2, in1=m2, op=ALU.subtract)
            sd = small.tile([128, 1], F32, tag="fin")
            nc.scalar.activation(out=sd, in_=var, func=AF.Sqrt, bias=eps_sb,
                                 scale=1.0)
            inv = small.tile([128, 1], F32, tag="fin")
            nc.vector.reciprocal(out=inv, in_=sd)
            scale = small.tile([128, 1], F32, tag="fin")
            nc.vector.tensor_tensor(out=scale, in0=inv,
                                    in1=gamma_sb[:, ch:ch + 1], op=ALU.mult)
            negshift = small.tile([128, 1], F32, tag="fin")
            nc.vector.scalar_tensor_tensor(
                out=negshift, in0=mean, scalar=scale,
                in1=beta_sb[:, ch:ch + 1], op0=ALU.mult, op1=ALU.subtract)
            nc.vector.scalar_tensor_tensor(
                out=zt, in0=wy[ch][:, :], scalar=scale, in1=x_bf[ch][:, :],
                op0=ALU.mult, op1=ALU.add)
            mx = small.tile([128, 1], F32, tag="fin")
            nc.vector.tensor_reduce(out=mx, in_=zt, axis=AX.X, op=ALU.max)
            nc.vector.tensor_tensor(out=out_sb[:, ch:ch + 1], in0=mx,
                                    in1=negshift, op=ALU.subtract)
        for ch in range(CCH):
            nc.sync.dma_start(
                out=out_d[ch, :].rearrange("(p one) -> p one", one=1),
                in_=out_sb[:, ch:ch + 1])

    nc.compile()
    return nc


def kernel(**inputs):
    x = np.ascontiguousarray(inputs["x"], dtype=np.float32)      # (8, 256, 64, 64)
    Wt = np.asarray(inputs["Wt"], dtype=np.float32)
    bt = np.asarray(inputs["bt"], dtype=np.float32)
    Wp = np.asarray(inputs["Wp"], dtype=np.float32)
    Wg = np.asarray(inputs["Wg"], dtype=np.float32)
    Ww = np.asarray(inputs["Ww"], dtype=np.float32)
    gamma = np.asarray(inputs["gamma"], dtype=np.float32)
    beta = np.asarray(inputs["beta"], dtype=np.float32)

    if "nc" not in _CACHE:
        _CACHE["nc"] = _build()
    nc = _CACHE["nc"]

    try:
        import ml_dtypes
        bf = ml_dtypes.bfloat16
    except ImportError:
        import jax.numpy as jnp
        bf = jnp.bfloat16

    shared = {
        "WtT": np.ascontiguousarray(Wt.T),
        "WpT": np.ascontiguousarray(Wp.T),
        "WgT": np.ascontiguousarray(Wg.T),
        "WwT": np.ascontiguousarray(Ww.T),
        "bt": np.ascontiguousarray(bt.reshape(CI, 1)),
        "gamma": np.ascontiguousarray(gamma.reshape(CCH, 128).T),
        "beta": np.ascontiguousarray(beta.reshape(CCH, 128).T),
    }
    in_maps = [dict(shared, x=np.ascontiguousarray(x[b].reshape(C, N)))
               for b in range(B)]
    import os
    trace = bool(int(os.environ.get("KERNEL_TRACE", "0")))
    res = run_bass_kernel_spmd(nc, in_maps, core_ids=list(range(B)), trace=trace)
    _LAST["res"] = res
    out = np.stack([np.asarray(res.results[b]["out"]).astype(np.float32).reshape(C)
                    for b in range(B)])
    return out.reshape(B, C, 1, 1).astype(np.float32)


if __name__ == "__main__":
    pass
